# revision 15
# baseline (speedup 1.0000x reference)
"""Trainium2 Bass kernel for nn_BertClassifier span-pair classifier.

Math (reference):
  vecs = hidden[:, 1:T+1, :]                                   [B,T,D]
  feat[b,i,j] = [vecs[b,i], vecs[b,j], ind[b,i,j]]             [2D+1]
  h   = relu(feat @ W1 + b1)                                   [B,T,T,H]
  out = h @ W2 + b2                                            [B,T,T,L]
  out = where(span_avail >= 1, out, 0)
  y   = log_softmax(out.reshape(B, T*T, L), axis=1)

Factorization: h[b,i,j] = relu(A[b,i] + C[b,j] + b1 + ind[b,i,j]*wl)
with A = vecs @ W1[:D], C = vecs @ W1[D:2D], wl = W1[2D].

Sharding: 8 cores, core c = (b = c//2, parity p = c%2); core handles rows
i = p, p+2, ..., p+126 of batch b (IH=64 rows x T=128 cols = 8192 pairs).

Engine strategy (vs the elementwise-heavy v1): the whole per-pair
pre-activation assembly runs on the TENSOR engine as fp8 DoubleRow
matmuls accumulating in PSUM -- one matmul per (quad of 4 i-rows,
128-wide h-chunk):
  k-tile 0: CTT^T @ Jsel        (broadcast C over i via a j-selector)
  k-tile 1: A2W^T @ KW_q        (row-select A+b1; + wl*ind via a wind row;
                                 + a -240*notavail row that drives masked
                                 pairs so negative relu yields exact 0)
The elementwise engines then do a single relu pass psum -> fp8 st tile,
and GEMM2 is 3 fp8 DoubleRow matmuls (K=256 each) + one bf16 matmul that
carries the 2-unit H-appendage, the b2*avail rank-1 term, so the final
value IS the GEMM2 psum (val = psum/32): no separate bias/mask pass.
log_softmax: exp+accum per quad on Act, AllReduce of the [L,1] sums over
the batch pair, y = val - ln(S) as an in-place 4x-mode TensorScalar on
the bf16 val buffer, stored via 2 DMAs.
"""
import sys
import dataclasses
from contextlib import ExitStack

sys.path.insert(0, "/opt/trn_rl_repo")

import numpy as np

import concourse.bass as bass
import concourse.tile as tile
from concourse import bacc, bass_utils, mybir
from concourse.masks import make_identity

B, T, D, H, L = 4, 128, 768, 770, 40
IH = T // 2         # 64 local rows per core
N_CORES = 8
QUAD = 4            # i-rows per asm psum group
NQ = IH // QUAD     # 16 quads
DRC = 3             # DoubleRow d-chunks (256 each) in GEMM1
HCC = 3             # DoubleRow h-chunks (256 each) in GEMM2
HP = 896            # h padded (7*128); cols 768:770 = appendage, rest 0
F32 = mybir.dt.float32
BF16 = mybir.dt.bfloat16
FP8 = mybir.dt.float8e4
W1SCALE = 16.0      # W1 shipped fp8 pre-scaled by 16
ACSCALE = 8.0       # A/C stored in fp8 pre-scaled by 8
W2SCALE = 32.0      # W2 shipped fp8 pre-scaled by 32
LP = 64             # L padded for DoubleRow weight tiles
MNEG = -240.0       # fp8e4m3 max-magnitude; kills masked pairs pre-relu


def _ap(ap_, dims, offset_elems=0):
    """AP with explicit free-dim [step, count] pairs on ap_'s partitions."""
    return dataclasses.replace(
        ap_, ap=[ap_.ap[0]] + [list(d) for d in dims],
        offset=ap_.offset + offset_elems)


def _bcast_src(dram, parts, cols, offset=0):
    """DRAM source AP replicating a row slice onto `parts` partitions."""
    return dataclasses.replace(
        dram.ap(), ap=[[0, parts], [1, cols]], offset=offset)


def build_program(timing_mode=False):
    """timing_mode=True: single-core variant, AllReduce replaced by a local
    DRAM->DRAM copy so the cost-model timeline simulator can run it."""
    nc = bacc.Bacc("TRN2", target_bir_lowering=False, debug=False,
                   num_devices=N_CORES)
    nc._timing_mode = timing_mode

    d_vecst = nc.dram_tensor("vecst", [128, DRC, 2, IH + T], FP8,
                             kind="ExternalInput")
    d_w1adr = nc.dram_tensor("w1adr", [128, DRC, 2, HP], FP8,
                             kind="ExternalInput")
    d_w1bdr = nc.dram_tensor("w1bdr", [128, DRC, 2, HP], FP8,
                             kind="ExternalInput")
    d_w2dr = nc.dram_tensor("w2dr", [128, HCC, 2, LP], FP8,
                            kind="ExternalInput")
    d_app3 = nc.dram_tensor("app3", [3, LP], BF16, kind="ExternalInput")
    d_asmrhs = nc.dram_tensor("asmrhs", [128, NQ + 1, 512], FP8,
                              kind="ExternalInput")
    d_alcw = nc.dram_tensor("alcw", [64, HP], FP8, kind="ExternalInput")
    d_b1row = nc.dram_tensor("b1row", [HP], BF16, kind="ExternalInput")
    d_windih = nc.dram_tensor("windih", [IH, 2 * T], BF16,
                              kind="ExternalInput")
    d_st6m = nc.dram_tensor("st6m", [IH * T], BF16, kind="ExternalInput")
    d_sel6 = nc.dram_tensor("sel6", [2, 2 * IH], BF16, kind="ExternalInput")
    d_out = nc.dram_tensor("out", [L, IH * T], BF16, kind="ExternalOutput")

    with tile.TileContext(nc) as tc, ExitStack() as stack:
        _build_tile(stack, tc, nc, d_vecst, d_w1adr, d_w1bdr, d_w2dr,
                    d_app3, d_asmrhs, d_alcw, d_b1row, d_windih, d_st6m,
                    d_sel6, d_out)
    nc.compile()
    return nc


def _build_tile(stack, tc, nc, d_vecst, d_w1adr, d_w1bdr, d_w2dr, d_app3,
                d_asmrhs, d_alcw, d_b1row, d_windih, d_st6m, d_sel6, d_out):
    Act = mybir.ActivationFunctionType
    Alu = mybir.AluOpType
    DR = mybir.MatmulPerfMode.DoubleRow
    W = IH + T

    const = stack.enter_context(tc.tile_pool(name="const", bufs=1))
    persist = stack.enter_context(tc.tile_pool(name="persist", bufs=1))

    ident = const.tile([128, 128], F32)
    make_identity(nc, ident[:])
    # PE warm-up: ~3us of continuous work to reach the fast p-state
    with tc.tile_pool(name="warm", bufs=2, space="PSUM") as warmp:
        for _ in range(18):
            wt = warmp.tile([128, 128], F32, tag="w")
            nc.tensor.transpose(wt[:], ident[:], ident[:])

    # warm the Exp/Ln activation tables
    dummy = const.tile([1, 2], F32)
    nc.vector.memset(dummy[:, 0:1], 1.0)
    nc.scalar.activation(dummy[:, 1:2], dummy[:, 0:1], Act.Ln)

    # ---- persistent tiles ----
    vT = persist.tile([128, DRC, 2, W], FP8)
    w1a = persist.tile([128, DRC, 2, HP], FP8)
    w1b = persist.tile([128, DRC, 2, HP], FP8)
    w2sb = persist.tile([128, HCC, 2, LP], FP8)
    app3sb = persist.tile([3, LP], BF16)
    asmrhs = persist.tile([128, NQ + 1, 512], FP8)
    ALC = persist.tile([128, 2, HP], FP8)       # [:,0,:]=8C ; [:,1,:]=8A+8b1
    b1rep = persist.tile([64, HP], BF16)        # 8*b1 replicated
    windih = persist.tile([IH, 2, T], BF16)     # 8*wl6[h']*ind - 240*notavail
    sel6 = const.tile([2, 2, IH], BF16)
    st6 = persist.tile([3, IH * T], BF16)       # appendage rows + avail row
    A6col = persist.tile([IH, 2], F32)
    c6sb = persist.tile([2, T], BF16)
    valP = persist.tile([LP, IH * T], BF16)
    Scols = persist.tile([LP, NQ], F32)

    # ---- input DMAs in descending criticality (HWDGE serializes) ----
    HB = 512  # first h-column block (psum chain 0)

    def col_dma(eng, sbuf_tile, dram, c0, c1):
        eng.dma_start(
            _ap(sbuf_tile[:], [[2 * HP, DRC], [HP, 2], [1, c1 - c0]],
                offset_elems=c0),
            dataclasses.replace(
                dram.ap(),
                ap=[[DRC * 2 * HP, 128], [2 * HP, DRC], [HP, 2],
                    [1, c1 - c0]], offset=c0))

    col_dma(nc.sync, w1b, d_w1bdr, 0, HB)
    nc.scalar.dma_start(vT[:], d_vecst.ap())
    nc.sync.dma_start(
        _ap(asmrhs[:], [[512, 5], [1, 512]]),
        dataclasses.replace(d_asmrhs.ap(),
                            ap=[[(NQ + 1) * 512, 128], [512, 5], [1, 512]]))
    col_dma(nc.scalar, w1a, d_w1adr, 0, HB)
    col_dma(nc.sync, w1b, d_w1bdr, HB, HP)
    nc.scalar.dma_start(b1rep[:], _bcast_src(d_b1row, 64, HP))
    col_dma(nc.sync, w1a, d_w1adr, HB, HP)
    nc.scalar.dma_start(ALC[64:128, 1, :], d_alcw.ap())
    nc.sync.dma_start(w2sb[:], d_w2dr.ap())
    nc.scalar.dma_start(app3sb[:], d_app3.ap())
    nc.sync.dma_start(windih[:], d_windih.ap().rearrange(
        "i (a t) -> i a t", a=2))
    nc.scalar.dma_start(sel6[:], d_sel6.ap().rearrange(
        "a (b i) -> a b i", b=2))
    nc.sync.dma_start(st6[2:3, :], dataclasses.replace(
        d_st6m.ap(), ap=[[0, 1], [1, IH * T]]))
    nc.scalar.dma_start(
        _ap(asmrhs[:], [[512, NQ - 4], [1, 512]], offset_elems=5 * 512),
        dataclasses.replace(d_asmrhs.ap(),
                            ap=[[(NQ + 1) * 512, 128], [512, NQ - 4],
                                [1, 512]], offset=5 * 512))

    # ---- GEMM1: DoubleRow chains, 2 column blocks each ----
    with tc.tile_pool(name="g1p", bufs=1, space="PSUM") as g1p:
        psC0 = g1p.tile([128, HB], F32, tag="c0")
        psC1 = g1p.tile([128, HP - HB], F32, tag="c1")
        psA0 = g1p.tile([64, HB], F32, tag="a0")
        psA1 = g1p.tile([64, HP - HB], F32, tag="a1")
        psC6 = g1p.tile([2, T], F32, tag="c6")

        def g1_chain(ps, wtile, vcols, c0, c1):
            for c in range(DRC):
                nc.tensor.matmul(
                    ps[:],
                    _ap(vT[:], [[W, 2], [1, vcols[1] - vcols[0]]],
                        offset_elems=c * 2 * W + vcols[0]),
                    _ap(wtile[:], [[HP, 2], [1, c1 - c0]],
                        offset_elems=c * 2 * HP + c0),
                    start=(c == 0), stop=(c == DRC - 1), perf_mode=DR)

        g1_chain(psC0, w1b, (IH, W), 0, HB)
        # C copy-outs feed the asm matmuls: emit early, split engines
        nc.vector.tensor_scalar(ALC[:, 0, 0:HB], psC0[:],
                                ACSCALE / W1SCALE, None, Alu.mult)
        g1_chain(psC1, w1b, (IH, W), HB, HP)
        nc.scalar.activation(ALC[:, 0, HB:HP], psC1[:], Act.Identity,
                             scale=ACSCALE / W1SCALE)
        g1_chain(psA0, w1a, (0, IH), 0, HB)
        nc.vector.scalar_tensor_tensor(ALC[0:64, 1, 0:HB], psA0[:],
                                       ACSCALE / W1SCALE, b1rep[:, 0:HB],
                                       Alu.mult, Alu.add)
        g1_chain(psA1, w1a, (0, IH), HB, HP)
        nc.vector.scalar_tensor_tensor(ALC[0:64, 1, HB:HP], psA1[:],
                                       ACSCALE / W1SCALE, b1rep[:, HB:HP],
                                       Alu.mult, Alu.add)
        # appendage A columns in fp32 (ptr-scalar source for the app pass)
        nc.vector.scalar_tensor_tensor(A6col[:], psA1[:, 768 - HB:770 - HB],
                                       ACSCALE / W1SCALE,
                                       b1rep[:, 768:770], Alu.mult, Alu.add)
        # appendage C row: psum [2, T] via swapped operands
        for c in range(DRC):
            nc.tensor.matmul(
                psC6[:],
                _ap(w1b[:], [[HP, 2], [1, 2]],
                    offset_elems=c * 2 * HP + 768),
                _ap(vT[:], [[W, 2], [1, T]], offset_elems=c * 2 * W + IH),
                start=(c == 0), stop=(c == DRC - 1), perf_mode=DR)
        nc.scalar.activation(c6sb[:], psC6[:], Act.Identity,
                             scale=ACSCALE / W1SCALE)

    # ---- appendage h=768..769 over the [i, j] grid, then collapse ----
    with tc.tile_pool(name="appp", bufs=1, space="PSUM") as appp, \
         tc.tile_pool(name="apps", bufs=1) as apool:
        for h in range(2):
            cjx = appp.tile([IH, T], F32, tag=f"cj{h}")
            nc.tensor.matmul(cjx[:], sel6[:, h, :], c6sb[:],
                             start=True, stop=True)
            tmp = apool.tile([IH, T], BF16, tag=f"apt{h}")
            nc.vector.scalar_tensor_tensor(tmp[:], windih[:, h, :],
                                           A6col[:, h:h + 1], cjx[:],
                                           Alu.add, Alu.add)
            nc.vector.tensor_scalar(tmp[:], tmp[:], 0.0, 1.0 / ACSCALE,
                                    Alu.max, Alu.mult)
            nc.sync.dma_start(
                st6[h:h + 1, :].rearrange("a (i j) -> a i j", i=IH),
                tmp[:])

    # ---- main loop over quads ----
    asmp = stack.enter_context(tc.tile_pool(name="asmp", bufs=1,
                                            space="PSUM"))
    g2p = stack.enter_context(tc.tile_pool(name="g2p", bufs=2,
                                           space="PSUM"))
    stp = stack.enter_context(tc.tile_pool(name="stp", bufs=3))
    scrp = stack.enter_context(tc.tile_pool(name="scr", bufs=2))

    # relus split DVE/Act (Pool = GPSIMD cannot read PSUM on hw): DVE gets
    # tile0 + tile1[:XS], Act gets tile1[XS:] + tile2 + the exp; the val
    # copy is a PSUM->SBUF DMA so no compute engine pays for it.
    XS = 764
    pend = []
    for q in range(NQ):
        rhs_ap = _ap(asmrhs[:], [[(1 + q) * 512, 2], [1, 512]])
        asmts = []
        for t in range(HCC):
            ps = asmp.tile([128, 1024], F32, tag=f"a{t}")
            for u in range(2):
                hc = 2 * t + u
                nc.tensor.matmul(
                    ps[:, u * 512:(u + 1) * 512],
                    ALC[:, :, hc * 128:(hc + 1) * 128],
                    rhs_ap, start=True, stop=True, perf_mode=DR)
            asmts.append(ps)

        st = stp.tile([128, HCC, 2, 512], FP8, tag="st")
        nc.vector.tensor_scalar(
            _ap(st[:], [[1, 1024]]), asmts[0][:], 0.0, 1.0 / ACSCALE,
            Alu.max, Alu.mult)
        nc.vector.tensor_scalar(
            _ap(st[:], [[1, XS]], offset_elems=1024), asmts[1][:, 0:XS],
            0.0, 1.0 / ACSCALE, Alu.max, Alu.mult)
        nc.scalar.activation(
            _ap(st[:], [[1, 1024 - XS]], offset_elems=1024 + XS),
            asmts[1][:, XS:1024], Act.Relu, scale=1.0 / ACSCALE)
        nc.scalar.activation(
            _ap(st[:], [[1, 1024]], offset_elems=2048), asmts[2][:],
            Act.Relu, scale=1.0 / ACSCALE)

        gp = g2p.tile([LP, 512], F32, tag="g2")
        nc.tensor.matmul(gp[:], app3sb[:],
                         st6[:, q * 512:(q + 1) * 512],
                         start=True, stop=False)
        for c in range(HCC):
            nc.tensor.matmul(
                gp[:], w2sb[:, c, :, :],
                _ap(st[:], [[512, 2], [1, 512]], offset_elems=c * 1024),
                start=False, stop=(c == HCC - 1), perf_mode=DR)

        pend.append((gp, q))
        if len(pend) > 2:
            _emit_val(nc, pend.pop(0), valP, Scols, scrp)
    while pend:
        _emit_val(nc, pend.pop(0), valP, Scols, scrp)

    # ---- AllReduce of exp-sums, LSE, subtract, store ----
    S_col = persist.tile([LP, 1], F32)
    nc.vector.tensor_reduce(S_col[:], Scols[:], mybir.AxisListType.X,
                            Alu.add)
    with tc.tile_pool(name="dram", bufs=1, space="DRAM") as dram:
        cin = dram.tile([LP, 1], F32)
        cout = dram.tile([LP, 1], F32)
        nc.sync.dma_start(cin[:], S_col[:])
        if getattr(nc, "_timing_mode", False):
            nc.sync.dma_start(cout[:], cin[:])
        else:
            nc.gpsimd.collective_compute(
                "AllReduce", Alu.add,
                replica_groups=[[2 * b, 2 * b + 1] for b in range(B)],
                ins=[cin.opt()], outs=[cout.opt()],
            )
        S_sb = persist.tile([LP, 1], F32)
        nc.sync.dma_start(S_sb[:], cout[:])

    lsecol = persist.tile([LP, 1], F32)
    nc.scalar.activation(lsecol[:], S_sb[:], Act.Ln)
    neg_lse = persist.tile([LP, 1], F32)
    nc.vector.tensor_scalar(neg_lse[:], lsecol[:], -1.0, None, Alu.mult)

    # in-place y = val - LSE: DVE runs 4x on the bf16 buffer, Act/Pool
    # take smaller slices; stores pipelined on 2 queues
    dmas = [nc.sync, nc.scalar]
    cuts = [0, 5440, 7168, IH * T]     # dve(4x), act, pool
    for tci in range(3):
        sl = slice(cuts[tci], cuts[tci + 1])
        if tci == 0:
            nc.vector.tensor_scalar(valP[:, sl], valP[:, sl], lsecol[:],
                                    None, Alu.subtract)
        elif tci == 1:
            nc.scalar.activation(valP[:, sl], valP[:, sl], Act.Identity,
                                 bias=neg_lse[:])
        else:
            nc.gpsimd.tensor_scalar(valP[:, sl], valP[:, sl], lsecol[:],
                                    None, Alu.subtract)
        dmas[tci % 2].dma_start(d_out.ap()[:, sl], valP[0:L, sl])


def _emit_val(nc, item, valP, Scols, scrp):
    """Deferred per-quad tail: val copy (psum/32 -> bf16) and exp+accum,
    both on Act; emitted 2 quads late to avoid head-of-line blocking."""
    Act = mybir.ActivationFunctionType
    gp, q = item
    sl = slice(q * 512, (q + 1) * 512)
    nc.scalar.activation(valP[:, sl], gp[:], Act.Identity,
                         scale=1.0 / W2SCALE)
    scr = scrp.tile([LP, 512], BF16, tag="scr")
    nc.scalar.activation(scr[:], gp[:], Act.Exp, scale=1.0 / W2SCALE,
                         accum_out=Scols[:, q:q + 1])


_NC_CACHE = {}


def _get_program():
    if "nc" not in _NC_CACHE:
        _NC_CACHE["nc"] = build_program()
    return _NC_CACHE["nc"]


def make_in_maps(hidden, W1, b1, W2, b2, pred_spans, span_avail):
    """Build the 8 per-core input dicts (all numpy)."""
    import ml_dtypes
    FP8NP = ml_dtypes.float8_e4m3   # mybir float8e4 = IEEE e4m3 (max 240)
    BF16NP = ml_dtypes.bfloat16
    hidden = np.asarray(hidden, np.float32)
    W1 = np.asarray(W1, np.float32)
    b1 = np.asarray(b1, np.float32)
    W2 = np.asarray(W2, np.float32)
    b2 = np.asarray(b2, np.float32)
    pred_spans = np.asarray(pred_spans).astype(np.int64)
    span_avail = np.asarray(span_avail).astype(np.int32)

    vecs = hidden[:, 1:T + 1, :]                      # [B,T,D]

    def drpack(arr, cols):
        # [768, cols] -> [128, DRC, 2, cols] with d = (2c+t)*128+p
        return np.ascontiguousarray(
            arr.reshape(DRC, 2, 128, cols).transpose(2, 0, 1, 3))

    w1ap = np.zeros((D, HP), np.float32)
    w1ap[:, :H] = W1[:D] * W1SCALE
    w1bp = np.zeros((D, HP), np.float32)
    w1bp[:, :H] = W1[D:2 * D] * W1SCALE
    w1adr = drpack(w1ap, HP).astype(FP8NP)
    w1bdr = drpack(w1bp, HP).astype(FP8NP)
    w2p = np.zeros((768, LP), np.float32)
    w2p[:, :L] = W2[:768] * W2SCALE
    w2dr = drpack(w2p, LP).astype(FP8NP)
    app3 = np.zeros((3, LP), np.float32)
    app3[:, :L] = np.concatenate([W2[768:770] * W2SCALE,
                                  (b2 * W2SCALE)[None, :]], 0)
    app3 = app3.astype(BF16NP)
    wl = W1[2 * D]                                    # [H]

    b1p = np.zeros((HP,), np.float32)
    b1p[:H] = b1
    b1row = (b1p * ACSCALE).astype(BF16NP)

    alcw = np.zeros((64, HP), np.float32)
    alcw[0, :H] = wl * ACSCALE
    alcw[1, :] = MNEG
    alcw = alcw.astype(FP8NP)

    sel6 = np.zeros((2, 2, IH), np.float32)
    sel6[0, 0] = 1.0
    sel6[1, 1] = 1.0
    sel6 = sel6.reshape(2, 2 * IH).astype(BF16NP)

    jsel = np.zeros((128, 512), np.float32)
    for k in range(QUAD):
        jsel[:, k * 128:(k + 1) * 128] = np.eye(128)

    jj = np.arange(T)[None, :]
    in_maps = []
    for c in range(N_CORES):
        b, p = c // 2, c % 2
        rows = np.arange(p, T, 2)
        s0, e0 = int(pred_spans[b, 0]), int(pred_spans[b, 1])
        ii = rows[:, None]
        inside = (s0 <= ii) & (ii <= jj) & (jj <= e0)
        full = (ii == s0) & (jj == e0)
        ind = inside.astype(np.float32) + full.astype(np.float32)  # [IH,T]
        avail = (span_avail[rows] >= 1).astype(np.float32)         # [IH,T]
        notav = 1.0 - avail

        asmrhs = np.zeros((128, NQ + 1, 512), np.float32)
        asmrhs[:, 0, :] = jsel
        for q in range(NQ):
            blk = asmrhs[:, 1 + q, :]
            for k in range(QUAD):
                blk[4 * q + k, k * 128:(k + 1) * 128] = 1.0
                blk[64, k * 128:(k + 1) * 128] = ind[4 * q + k]
                blk[65, k * 128:(k + 1) * 128] = notav[4 * q + k]
        asmrhs = asmrhs.astype(FP8NP)

        windih = np.stack(
            [ACSCALE * wl[768 + hh] * ind + MNEG * notav
             for hh in range(2)], axis=1)                 # [IH, 2, T]
        windih = windih.reshape(IH, 2 * T).astype(BF16NP)

        vecst = np.concatenate([vecs[b, p::2], vecs[b]], 0).T  # [768, 192]
        vecst = drpack(vecst, IH + T).astype(FP8NP)

        in_maps.append({
            "vecst": vecst, "w1adr": w1adr, "w1bdr": w1bdr,
            "w2dr": w2dr, "app3": app3,
            "asmrhs": np.ascontiguousarray(asmrhs),
            "alcw": alcw, "b1row": b1row,
            "windih": np.ascontiguousarray(windih),
            "st6m": avail.reshape(-1).astype(BF16NP),
            "sel6": sel6,
        })
    return in_maps


def unshard(results):
    """results: list of 8 dicts with 'out' [L, IH*T] -> full [B, T*T, L]."""
    full = np.empty((B, T, T, L), np.float32)
    for c in range(N_CORES):
        b, p = c // 2, c % 2
        arr = np.asarray(results[c]["out"], np.float32)   # [L, IH*T]
        full[b, p::2] = arr.reshape(L, IH, T).transpose(1, 2, 0)
    return full.reshape(B, T * T, L)


def kernel(hidden, W1, b1, W2, b2, pred_spans, span_avail, token_num):
    assert int(np.asarray(token_num)) == T, "kernel specialized for T=128"
    in_maps = make_in_maps(hidden, W1, b1, W2, b2, pred_spans, span_avail)
    nc = _get_program()
    res = bass_utils.run_bass_kernel_spmd(
        nc, in_maps, core_ids=list(range(N_CORES)))
    return unshard(res.results)


# revision 24
# speedup vs baseline: 1.2345x; 1.2345x over previous
"""Trainium2 Bass kernel for nn_BertClassifier span-pair classifier.

Math (reference):
  vecs = hidden[:, 1:T+1, :]                                   [B,T,D]
  feat[b,i,j] = [vecs[b,i], vecs[b,j], ind[b,i,j]]             [2D+1]
  h   = relu(feat @ W1 + b1)                                   [B,T,T,H]
  out = h @ W2 + b2                                            [B,T,T,L]
  out = where(span_avail >= 1, out, 0)
  y   = log_softmax(out.reshape(B, T*T, L), axis=1)

Factorization: h[b,i,j] = relu(A[b,i] + C[b,j] + b1 + ind[b,i,j]*wl)
with A = vecs @ W1[:D], C = vecs @ W1[D:2D], wl = W1[2D].

Sharding: 8 cores, core c = (b = c//2, parity p = c%2); core handles rows
i = p, p+2, ..., p+126 of batch b (IH=64 rows x T=128 cols = 8192 pairs).

Engine strategy (vs the elementwise-heavy v1): the whole per-pair
pre-activation assembly runs on the TENSOR engine as fp8 DoubleRow
matmuls accumulating in PSUM -- one matmul per (quad of 4 i-rows,
128-wide h-chunk):
  k-tile 0: CTT^T @ Jsel        (broadcast C over i via a j-selector)
  k-tile 1: A2W^T @ KW_q        (row-select A+b1; + wl*ind via a wind row;
                                 + a -240*notavail row that drives masked
                                 pairs so negative relu yields exact 0)
The elementwise engines then do a single relu pass psum -> fp8 st tile,
and GEMM2 is 3 fp8 DoubleRow matmuls (K=256 each) + one bf16 matmul that
carries the 2-unit H-appendage, the b2*avail rank-1 term, so the final
value IS the GEMM2 psum (val = psum/32): no separate bias/mask pass.
log_softmax: exp+accum per quad on Act, AllReduce of the [L,1] sums over
the batch pair, y = val - ln(S) as an in-place 4x-mode TensorScalar on
the bf16 val buffer, stored via 2 DMAs.
"""
import sys
import dataclasses
from contextlib import ExitStack

sys.path.insert(0, "/opt/trn_rl_repo")

import numpy as np

import concourse.bass as bass
import concourse.tile as tile
from concourse import bacc, bass_utils, mybir
from concourse.masks import make_identity

B, T, D, H, L = 4, 128, 768, 770, 40
IH = T // 2         # 64 local rows per core
N_CORES = 8
QUAD = 4            # i-rows per asm psum group
NQ = IH // QUAD     # 16 quads
DRC = 3             # DoubleRow d-chunks (256 each) in GEMM1
HCC = 3             # DoubleRow h-chunks (256 each) in GEMM2
HM = 768            # main h columns (6*128); appendage shipped separately
F32 = mybir.dt.float32
BF16 = mybir.dt.bfloat16
FP8 = mybir.dt.float8e4
W1SCALE = 16.0      # W1 shipped fp8 pre-scaled by 16
ACSCALE = 8.0       # A/C stored in fp8 pre-scaled by 8
W2SCALE = 32.0      # W2 shipped fp8 pre-scaled by 32
LP = 64             # L padded for DoubleRow weight tiles
MNEG = -240.0       # fp8e4m3 max-magnitude; kills masked pairs pre-relu


def _ap(ap_, dims, offset_elems=0):
    """AP with explicit free-dim [step, count] pairs on ap_'s partitions."""
    return dataclasses.replace(
        ap_, ap=[ap_.ap[0]] + [list(d) for d in dims],
        offset=ap_.offset + offset_elems)


def _bcast_src(dram, parts, cols, offset=0):
    """DRAM source AP replicating a row slice onto `parts` partitions."""
    return dataclasses.replace(
        dram.ap(), ap=[[0, parts], [1, cols]], offset=offset)


def build_program(timing_mode=False):
    """timing_mode=True: single-core variant, AllReduce replaced by a local
    DRAM->DRAM copy so the cost-model timeline simulator can run it."""
    nc = bacc.Bacc("TRN2", target_bir_lowering=False, debug=False,
                   num_devices=N_CORES)
    nc._timing_mode = timing_mode

    d_vecst = nc.dram_tensor("vecst", [128, DRC, 2, IH + T], FP8,
                             kind="ExternalInput")
    d_w1adr = nc.dram_tensor("w1adr", [128, DRC, 2, HM], FP8,
                             kind="ExternalInput")
    d_w1bdr = nc.dram_tensor("w1bdr", [128, DRC, 2, HM], FP8,
                             kind="ExternalInput")
    d_w1app = nc.dram_tensor("w1app", [128, DRC, 2, 64], FP8,
                             kind="ExternalInput")
    d_w2dr = nc.dram_tensor("w2dr", [128, HCC, 2, LP], FP8,
                            kind="ExternalInput")
    d_app3 = nc.dram_tensor("app3", [3, LP], BF16, kind="ExternalInput")
    d_asmrhs = nc.dram_tensor("asmrhs", [128, NQ + 1, 512], FP8,
                              kind="ExternalInput")
    d_alcw = nc.dram_tensor("alcw", [64, HM], FP8, kind="ExternalInput")
    d_b1row = nc.dram_tensor("b1row", [HM], BF16, kind="ExternalInput")
    d_windih = nc.dram_tensor("windih", [IH, 2 * T], BF16,
                              kind="ExternalInput")
    d_st6m = nc.dram_tensor("st6m", [IH * T], BF16, kind="ExternalInput")
    d_sel6 = nc.dram_tensor("sel6", [2, 2 * IH], BF16, kind="ExternalInput")
    d_out = nc.dram_tensor("out", [L, IH * T], BF16, kind="ExternalOutput")

    with tile.TileContext(nc) as tc, ExitStack() as stack:
        _build_tile(stack, tc, nc, d_vecst, d_w1adr, d_w1bdr, d_w1app,
                    d_w2dr, d_app3, d_asmrhs, d_alcw, d_b1row, d_windih,
                    d_st6m, d_sel6, d_out)
    nc.compile()
    return nc


def _build_tile(stack, tc, nc, d_vecst, d_w1adr, d_w1bdr, d_w1app, d_w2dr,
                d_app3, d_asmrhs, d_alcw, d_b1row, d_windih, d_st6m,
                d_sel6, d_out):
    Act = mybir.ActivationFunctionType
    Alu = mybir.AluOpType
    DR = mybir.MatmulPerfMode.DoubleRow
    W = IH + T

    const = stack.enter_context(tc.tile_pool(name="const", bufs=1))
    persist = stack.enter_context(tc.tile_pool(name="persist", bufs=1))

    ident = const.tile([128, 128], F32)
    make_identity(nc, ident[:])
    # PE warm-up: ~3us of continuous work to reach the fast p-state
    with tc.tile_pool(name="warm", bufs=2, space="PSUM") as warmp:
        for _ in range(18):
            wt = warmp.tile([128, 128], F32, tag="w")
            nc.tensor.transpose(wt[:], ident[:], ident[:])

    # warm the activation table: require Exp AND Ln up front so the table
    # chooser settles on natural_log_exp_and_others (covers Relu/Identity/
    # Exp/Ln) once, with no mid-loop reloads
    dummy = const.tile([1, 3], F32)
    nc.vector.memset(dummy[:, 0:1], 1.0)
    nc.scalar.activation(dummy[:, 1:2], dummy[:, 0:1], Act.Ln)
    nc.scalar.activation(dummy[:, 2:3], dummy[:, 0:1], Act.Exp)

    # ---- persistent tiles ----
    vT = persist.tile([128, DRC, 2, W], FP8)
    w1a = persist.tile([128, DRC, 2, HM], FP8)
    w1b = persist.tile([128, DRC, 2, HM], FP8)
    w1app = const.tile([128, DRC, 2, 64], FP8)
    w2sb = persist.tile([128, HCC, 2, LP], FP8)
    app3sb = persist.tile([3, LP], BF16)
    asmrhs = persist.tile([128, NQ + 1, 512], FP8)
    ALC = persist.tile([128, 2, HM], FP8)       # [:,0,:]=8C ; [:,1,:]=8A+8b1
    b1rep = persist.tile([64, HM], BF16)        # 8*b1 replicated
    windih = persist.tile([IH, 2, T], BF16)     # 8*wl6[h']*ind - 240*notavail
    sel6 = const.tile([2, 2, IH], BF16)
    st6 = persist.tile([3, IH * T], BF16)       # appendage rows + avail row
    A6col = persist.tile([IH, 2], F32)
    c6sb = persist.tile([2, T], BF16)
    valP = persist.tile([LP, IH * T], BF16)
    Scols = persist.tile([LP, NQ], F32)

    # ---- input DMAs in descending criticality (one serialized DMA
    # device in the model, so list order ~= arrival order) ----
    HB = 512  # first h-column block (psum chain 0)

    def col_dma(eng, sbuf_tile, dram, c0, c1):
        eng.dma_start(
            _ap(sbuf_tile[:], [[2 * HM, DRC], [HM, 2], [1, c1 - c0]],
                offset_elems=c0),
            dataclasses.replace(
                dram.ap(),
                ap=[[DRC * 2 * HM, 128], [2 * HM, DRC], [HM, 2],
                    [1, c1 - c0]], offset=c0))

    col_dma(nc.sync, w1b, d_w1bdr, 0, HB)
    nc.scalar.dma_start(vT[:], d_vecst.ap())
    nc.sync.dma_start(b1rep[:], _bcast_src(d_b1row, 64, HM))
    nc.scalar.dma_start(w1app[:], d_w1app.ap())
    nc.sync.dma_start(windih[:], d_windih.ap().rearrange(
        "i (a t) -> i a t", a=2))
    nc.scalar.dma_start(sel6[:], d_sel6.ap().rearrange(
        "a (b i) -> a b i", b=2))
    col_dma(nc.sync, w1a, d_w1adr, 0, HB)
    col_dma(nc.scalar, w1b, d_w1bdr, HB, HM)
    nc.sync.dma_start(ALC[64:128, 1, :], d_alcw.ap())
    col_dma(nc.scalar, w1a, d_w1adr, HB, HM)
    nc.sync.dma_start(
        _ap(asmrhs[:], [[512, 3], [1, 512]]),
        dataclasses.replace(d_asmrhs.ap(),
                            ap=[[(NQ + 1) * 512, 128], [512, 3], [1, 512]]))
    nc.scalar.dma_start(st6[2:3, :], dataclasses.replace(
        d_st6m.ap(), ap=[[0, 1], [1, IH * T]]))
    nc.sync.dma_start(w2sb[:], d_w2dr.ap())
    nc.scalar.dma_start(app3sb[:], d_app3.ap())
    nc.sync.dma_start(
        _ap(asmrhs[:], [[512, NQ - 2], [1, 512]], offset_elems=3 * 512),
        dataclasses.replace(d_asmrhs.ap(),
                            ap=[[(NQ + 1) * 512, 128], [512, NQ - 2],
                                [1, 512]], offset=3 * 512))

    # ---- GEMM1: DoubleRow chains, 2 column blocks each; the tiny
    # appendage chains (w1app) run first so the st6 path starts early ----
    with tc.tile_pool(name="g1p", bufs=1, space="PSUM") as g1p:
        psC0 = g1p.tile([128, HB], F32, tag="c0")
        psC1 = g1p.tile([128, HM - HB], F32, tag="c1")
        psA0 = g1p.tile([64, HB], F32, tag="a0")
        psA1 = g1p.tile([64, HM - HB], F32, tag="a1")
        psA6 = g1p.tile([64, 2], F32, tag="a6")
        psC6 = g1p.tile([2, T], F32, tag="c6")

        def g1_chain(ps, wtile, vcols, c0, c1):
            for c in range(DRC):
                nc.tensor.matmul(
                    ps[:],
                    _ap(vT[:], [[W, 2], [1, vcols[1] - vcols[0]]],
                        offset_elems=c * 2 * W + vcols[0]),
                    _ap(wtile[:], [[HM, 2], [1, c1 - c0]],
                        offset_elems=c * 2 * HM + c0),
                    start=(c == 0), stop=(c == DRC - 1), perf_mode=DR)

        # appendage C row [2, T] (swapped operands) and A cols [64, 2]
        for c in range(DRC):
            nc.tensor.matmul(
                psC6[:],
                _ap(w1app[:], [[64, 2], [1, 2]], offset_elems=c * 128),
                _ap(vT[:], [[W, 2], [1, T]], offset_elems=c * 2 * W + IH),
                start=(c == 0), stop=(c == DRC - 1), perf_mode=DR)
        for c in range(DRC):
            nc.tensor.matmul(
                psA6[:],
                _ap(vT[:], [[W, 2], [1, IH]], offset_elems=c * 2 * W),
                _ap(w1app[:], [[64, 2], [1, 2]], offset_elems=c * 128 + 32),
                start=(c == 0), stop=(c == DRC - 1), perf_mode=DR)
        nc.scalar.activation(c6sb[:], psC6[:], Act.Identity,
                             scale=ACSCALE / W1SCALE)
        nc.vector.tensor_scalar(A6col[:], psA6[:], ACSCALE / W1SCALE,
                                None, Alu.mult)

        g1_chain(psC0, w1b, (IH, W), 0, HB)
        # C copy-outs feed the asm matmuls: emit early, split engines
        nc.vector.tensor_scalar(ALC[:, 0, 0:HB], psC0[:],
                                ACSCALE / W1SCALE, None, Alu.mult)
        g1_chain(psA0, w1a, (0, IH), 0, HB)
        nc.vector.scalar_tensor_tensor(ALC[0:64, 1, 0:HB], psA0[:],
                                       ACSCALE / W1SCALE, b1rep[:, 0:HB],
                                       Alu.mult, Alu.add)
        g1_chain(psC1, w1b, (IH, W), HB, HM)
        nc.scalar.activation(ALC[:, 0, HB:HM], psC1[:], Act.Identity,
                             scale=ACSCALE / W1SCALE)
        g1_chain(psA1, w1a, (0, IH), HB, HM)
        nc.vector.scalar_tensor_tensor(ALC[0:64, 1, HB:HM], psA1[:],
                                       ACSCALE / W1SCALE, b1rep[:, HB:HM],
                                       Alu.mult, Alu.add)

    # ---- appendage h=768..769 over the [i, j] grid, then collapse ----
    with tc.tile_pool(name="appp", bufs=1, space="PSUM") as appp, \
         tc.tile_pool(name="apps", bufs=1) as apool:
        for h in range(2):
            cjx = appp.tile([IH, T], F32, tag=f"cj{h}")
            nc.tensor.matmul(cjx[:], sel6[:, h, :], c6sb[:],
                             start=True, stop=True)
            tmp = apool.tile([IH, T], BF16, tag=f"apt{h}")
            nc.vector.scalar_tensor_tensor(tmp[:], windih[:, h, :],
                                           A6col[:, h:h + 1], cjx[:],
                                           Alu.add, Alu.add)
            nc.vector.tensor_scalar(tmp[:], tmp[:], 0.0, 1.0 / ACSCALE,
                                    Alu.max, Alu.mult)
            nc.sync.dma_start(
                st6[h:h + 1, :].rearrange("a (i j) -> a i j", i=IH),
                tmp[:])

    # ---- main loop over quads ----
    asmp = stack.enter_context(tc.tile_pool(name="asmp", bufs=1,
                                            space="PSUM"))
    g2p = stack.enter_context(tc.tile_pool(name="g2p", bufs=2,
                                           space="PSUM"))
    stp = stack.enter_context(tc.tile_pool(name="stp", bufs=3))
    scrp = stack.enter_context(tc.tile_pool(name="scr", bufs=2))

    # relus split DVE/Act (Pool = GPSIMD cannot read PSUM on hw): DVE gets
    # tiles 0+1, Act tile2 + the val copy + exp.  PE emission is skewed
    # one quad -- asm(q+1) precedes GEMM2(q) in the queue -- so GEMM2's
    # wait on relu(q) never head-of-line-blocks the next quad's assembly.
    def emit_asm(q):
        rhs_ap = _ap(asmrhs[:], [[(1 + q) * 512, 2], [1, 512]])
        asmts = []
        for t in range(HCC):
            ps = asmp.tile([128, 1024], F32, tag=f"a{t}")
            for u in range(2):
                hc = 2 * t + u
                nc.tensor.matmul(
                    ps[:, u * 512:(u + 1) * 512],
                    ALC[:, :, hc * 128:(hc + 1) * 128],
                    rhs_ap, start=True, stop=True, perf_mode=DR)
            asmts.append(ps)
        return asmts

    def emit_relu(q, asmts):
        st = stp.tile([128, HCC, 2, 512], FP8, tag="st")
        nc.vector.tensor_scalar(
            _ap(st[:], [[1, 1024]]), asmts[0][:], 0.0, 1.0 / ACSCALE,
            Alu.max, Alu.mult)
        nc.vector.tensor_scalar(
            _ap(st[:], [[1, 1024]], offset_elems=1024), asmts[1][:],
            0.0, 1.0 / ACSCALE, Alu.max, Alu.mult)
        nc.scalar.activation(
            _ap(st[:], [[1, 1024]], offset_elems=2048), asmts[2][:],
            Act.Relu, scale=1.0 / ACSCALE)
        return st

    def emit_g2(q, st):
        gp = g2p.tile([LP, 512], F32, tag="g2")
        nc.tensor.matmul(gp[:], app3sb[:],
                         st6[:, q * 512:(q + 1) * 512],
                         start=True, stop=False)
        for c in range(HCC):
            nc.tensor.matmul(
                gp[:], w2sb[:, c, :, :],
                _ap(st[:], [[512, 2], [1, 512]], offset_elems=c * 1024),
                start=False, stop=(c == HCC - 1), perf_mode=DR)
        return gp

    pend = []
    prev = None                 # (q, st) awaiting GEMM2
    for q in range(NQ):
        asmts = emit_asm(q)
        if prev is not None:
            pend.append((emit_g2(prev[0], prev[1]), prev[0]))
        prev = (q, emit_relu(q, asmts))
        if len(pend) > 2:
            _emit_val(nc, pend.pop(0), valP, Scols, scrp)
    pend.append((emit_g2(prev[0], prev[1]), prev[0]))
    while pend:
        _emit_val(nc, pend.pop(0), valP, Scols, scrp)

    # ---- AllReduce of exp-sums, LSE, subtract, store ----
    S_col = persist.tile([LP, 1], F32)
    nc.vector.tensor_reduce(S_col[:], Scols[:], mybir.AxisListType.X,
                            Alu.add)
    with tc.tile_pool(name="dram", bufs=1, space="DRAM") as dram:
        cin = dram.tile([LP, 1], F32)
        cout = dram.tile([LP, 1], F32)
        nc.sync.dma_start(cin[:], S_col[:])
        if getattr(nc, "_timing_mode", False):
            nc.sync.dma_start(cout[:], cin[:])
        else:
            nc.gpsimd.collective_compute(
                "AllReduce", Alu.add,
                replica_groups=[[2 * b, 2 * b + 1] for b in range(B)],
                ins=[cin.opt()], outs=[cout.opt()],
            )
        S_sb = persist.tile([LP, 1], F32)
        nc.sync.dma_start(S_sb[:], cout[:])

    lsecol = persist.tile([LP, 1], F32)
    nc.scalar.activation(lsecol[:], S_sb[:], Act.Ln)
    neg_lse = persist.tile([LP, 1], F32)
    nc.vector.tensor_scalar(neg_lse[:], lsecol[:], -1.0, None, Alu.mult)

    # in-place y = val - LSE: DVE runs 4x on the bf16 buffer, Act/Pool
    # take smaller slices; stores pipelined on 2 queues
    dmas = [nc.sync, nc.scalar]
    cuts = [0, 5440, 7168, IH * T]     # dve(4x), act, pool
    for tci in range(3):
        sl = slice(cuts[tci], cuts[tci + 1])
        if tci == 0:
            nc.vector.tensor_scalar(valP[:, sl], valP[:, sl], lsecol[:],
                                    None, Alu.subtract)
        elif tci == 1:
            nc.scalar.activation(valP[:, sl], valP[:, sl], Act.Identity,
                                 bias=neg_lse[:])
        else:
            nc.gpsimd.tensor_scalar(valP[:, sl], valP[:, sl], lsecol[:],
                                    None, Alu.subtract)
        dmas[tci % 2].dma_start(d_out.ap()[:, sl], valP[0:L, sl])


def _emit_val(nc, item, valP, Scols, scrp):
    """Deferred per-quad tail: val copy (psum/32 -> bf16) and exp+accum,
    both on Act; emitted 2 quads late to avoid head-of-line blocking."""
    Act = mybir.ActivationFunctionType
    gp, q = item
    sl = slice(q * 512, (q + 1) * 512)
    nc.scalar.activation(valP[:, sl], gp[:], Act.Identity,
                         scale=1.0 / W2SCALE)
    scr = scrp.tile([LP, 512], BF16, tag="scr")
    nc.scalar.activation(scr[:], gp[:], Act.Exp, scale=1.0 / W2SCALE,
                         accum_out=Scols[:, q:q + 1])


_NC_CACHE = {}


def _get_program():
    if "nc" not in _NC_CACHE:
        _NC_CACHE["nc"] = build_program()
    return _NC_CACHE["nc"]


def make_in_maps(hidden, W1, b1, W2, b2, pred_spans, span_avail):
    """Build the 8 per-core input dicts (all numpy)."""
    import ml_dtypes
    FP8NP = ml_dtypes.float8_e4m3   # mybir float8e4 = IEEE e4m3 (max 240)
    BF16NP = ml_dtypes.bfloat16
    hidden = np.asarray(hidden, np.float32)
    W1 = np.asarray(W1, np.float32)
    b1 = np.asarray(b1, np.float32)
    W2 = np.asarray(W2, np.float32)
    b2 = np.asarray(b2, np.float32)
    pred_spans = np.asarray(pred_spans).astype(np.int64)
    span_avail = np.asarray(span_avail).astype(np.int32)

    vecs = hidden[:, 1:T + 1, :]                      # [B,T,D]

    def drpack(arr, cols):
        # [768, cols] -> [128, DRC, 2, cols] with d = (2c+t)*128+p
        return np.ascontiguousarray(
            arr.reshape(DRC, 2, 128, cols).transpose(2, 0, 1, 3))

    w1adr = drpack(W1[:D, :HM] * W1SCALE, HM).astype(FP8NP)
    w1bdr = drpack(W1[D:2 * D, :HM] * W1SCALE, HM).astype(FP8NP)
    # appendage weight cols: [b768, b769] at 0, [a768, a769] at 32
    # (DoubleRow Ldweights wants 32B-aligned k-tile strides/offsets)
    w1app_f = np.zeros((D, 64), np.float32)
    w1app_f[:, 0:2] = W1[D:2 * D, HM:H] * W1SCALE
    w1app_f[:, 32:34] = W1[:D, HM:H] * W1SCALE
    w1app = drpack(w1app_f, 64).astype(FP8NP)
    w2p = np.zeros((768, LP), np.float32)
    w2p[:, :L] = W2[:768] * W2SCALE
    w2dr = drpack(w2p, LP).astype(FP8NP)
    app3 = np.zeros((3, LP), np.float32)
    app3[:, :L] = np.concatenate([W2[768:770] * W2SCALE,
                                  (b2 * W2SCALE)[None, :]], 0)
    app3 = app3.astype(BF16NP)
    wl = W1[2 * D]                                    # [H]

    b1row = (b1[:HM] * ACSCALE).astype(BF16NP)

    alcw = np.zeros((64, HM), np.float32)
    alcw[0, :] = wl[:HM] * ACSCALE
    alcw[1, :] = MNEG
    alcw = alcw.astype(FP8NP)

    sel6 = np.zeros((2, 2, IH), np.float32)
    sel6[0, 0] = 1.0
    sel6[1, 1] = 1.0
    sel6 = sel6.reshape(2, 2 * IH).astype(BF16NP)

    jsel = np.zeros((128, 512), np.float32)
    for k in range(QUAD):
        jsel[:, k * 128:(k + 1) * 128] = np.eye(128)

    jj = np.arange(T)[None, :]
    in_maps = []
    for c in range(N_CORES):
        b, p = c // 2, c % 2
        rows = np.arange(p, T, 2)
        s0, e0 = int(pred_spans[b, 0]), int(pred_spans[b, 1])
        ii = rows[:, None]
        inside = (s0 <= ii) & (ii <= jj) & (jj <= e0)
        full = (ii == s0) & (jj == e0)
        ind = inside.astype(np.float32) + full.astype(np.float32)  # [IH,T]
        avail = (span_avail[rows] >= 1).astype(np.float32)         # [IH,T]
        notav = 1.0 - avail

        asmrhs = np.zeros((128, NQ + 1, 512), np.float32)
        asmrhs[:, 0, :] = jsel
        for q in range(NQ):
            blk = asmrhs[:, 1 + q, :]
            for k in range(QUAD):
                blk[4 * q + k, k * 128:(k + 1) * 128] = 1.0
                blk[64, k * 128:(k + 1) * 128] = ind[4 * q + k]
                blk[65, k * 128:(k + 1) * 128] = notav[4 * q + k]
        asmrhs = asmrhs.astype(FP8NP)

        # carries the appendage wl*ind, the -240 mask drive, AND 8*b1[768+h]
        windih = np.stack(
            [ACSCALE * (wl[768 + hh] * ind + b1[768 + hh]) + MNEG * notav
             for hh in range(2)], axis=1)                 # [IH, 2, T]
        windih = windih.reshape(IH, 2 * T).astype(BF16NP)

        vecst = np.concatenate([vecs[b, p::2], vecs[b]], 0).T  # [768, 192]
        vecst = drpack(vecst, IH + T).astype(FP8NP)

        in_maps.append({
            "vecst": vecst, "w1adr": w1adr, "w1bdr": w1bdr,
            "w1app": w1app, "w2dr": w2dr, "app3": app3,
            "asmrhs": np.ascontiguousarray(asmrhs),
            "alcw": alcw, "b1row": b1row,
            "windih": np.ascontiguousarray(windih),
            "st6m": avail.reshape(-1).astype(BF16NP),
            "sel6": sel6,
        })
    return in_maps


def unshard(results):
    """results: list of 8 dicts with 'out' [L, IH*T] -> full [B, T*T, L]."""
    full = np.empty((B, T, T, L), np.float32)
    for c in range(N_CORES):
        b, p = c // 2, c % 2
        arr = np.asarray(results[c]["out"], np.float32)   # [L, IH*T]
        full[b, p::2] = arr.reshape(L, IH, T).transpose(1, 2, 0)
    return full.reshape(B, T * T, L)


def kernel(hidden, W1, b1, W2, b2, pred_spans, span_avail, token_num):
    assert int(np.asarray(token_num)) == T, "kernel specialized for T=128"
    in_maps = make_in_maps(hidden, W1, b1, W2, b2, pred_spans, span_avail)
    nc = _get_program()
    res = bass_utils.run_bass_kernel_spmd(
        nc, in_maps, core_ids=list(range(N_CORES)))
    return unshard(res.results)
